# revision 35
# baseline (speedup 1.0000x reference)
"""Trainium2 Bass kernel for a 2-layer GCN encoder + edge dot-product decoder.

Math (matches the PyG-style reference):
    deg  = in-degree(dst)+1 (self loops), dinv = rsqrt(deg)
    A~[d,s] = dinv[s]*dinv[d] over edges+self-loops
    H1 = (A~ @ X) @ W1 + b1          (aggregate-first ordering)
    Z  = (A~ @ relu(H1) @ W2) + b2
    logits[e] = <Z[src_e], Z[dst_e]>

Distribution over 8 NeuronCores: nodes are LPT-assigned to (core, tile,
slot) buckets balancing per-bucket edge counts; edges partitioned by
dst-owner.  The scatter-sum runs on the Tensor Engine: per 128-edge
block a [128e x 128slot] S matrix (f16) left-multiplies the source
rows, accumulating in PSUM per dst tile.  Self-loops are one dedicated
"self block" per dst tile whose S is diag(dinv^2); in layer 2 its rhs
is the SBUF-resident h2 tile, so the self rows are never gathered.

Gathers (SWDGE descriptor generation on gpsimd ~2.5 ns/row is the hard
wall):
  - layer 1 reads x[src] via a host-staged edge-major f16 copy of x
    (xe), streamed with direct DMAs split over two HWDGE queues;
  - layer 2 gathers h2'[src] from the AllGathered tables with batched
    gpsimd dma_gather round-robined over 4 SWDGE queues (int16 indices);
  - the decoder gathers z[src] TRANSPOSED (feature-major zsT, <=512
    idxs per call), expands z[dst] with per-tile-run batched one-hot
    matmuls (lhsT = zloc), does one wide DVE multiply per run and
    reduces over features with a ones-vector matmul on the PE
    -> [1, 512] logit rows.
Each AllGather is split in 2 chunks into two Shared tables (table A =
tiles 0..31 on all cores = 32768 rows, table B = tiles 32..48 = 17408
rows) so chunk A overlaps the producer's tail and chunk B hides under
the consumer's table-A gathers.  (fp8 DoubleRow was tried for the
scatter + GEMMs and is numerically dead here: each fp8-quantized tensor
adds ~3.5% relative error that propagates linearly through the GEMM
chain -> ~6e-2 logits error vs the 2e-2 budget.)
"""

import os

if os.environ.get("JAX_PLATFORMS") == "cpu":
    os.environ.pop("JAX_PLATFORMS")

import numpy as np

from concourse import bass, bacc, mybir, bass_utils
import concourse.tile as tile

# ---------------------------------------------------------------- sizes
N_NODES = 50000
N_EDGES = 400000
D_IN, D_H, D_OUT = 600, 628, 64
C = 8
P = 128

NPC = N_NODES // C               # 6250 real nodes per core
TILES = -(-NPC // P)             # 49 dst tiles per core
NPAD = TILES * P                 # 6272 padded nodes per core
NS = C * NPAD                    # 50176 staged rows
TCHUNK = 32                      # AllGather chunk 1 = tiles [0, 32)
R1 = TCHUNK * P                  # 4096 local rows in chunk 1
NSA = C * R1                     # 32768 rows in table A (fits int16)
NSB = NS - NSA                   # 17408 rows in table B
GMAX = 8                         # blocks (1024 idxs) per gather batch
TGMAX = 4                        # transposed gather cap (512 idxs)
NQ = 4                           # SWDGE queues

F16 = mybir.dt.float16
F32 = mybir.dt.float32
I16 = mybir.dt.int16

KCH = [(0, 128), (128, 128), (256, 128), (384, 128), (512, 88)]
MCH = [(0, 128), (128, 128), (256, 128), (384, 128), (512, 116)]
GROUPS = [list(range(i, min(i + 3, TILES))) for i in range(0, TILES, 3)]
NGROUP_C1 = 11                   # groups 0..10 cover tiles 0..32 (>= 0..31)


def _wrap16(vals, nblocks):
    """[nblocks*128] -> wrapped int16 [128, nblocks*8] (index i at row i%16
    col i//16, replicated across the 8 groups of 16 partitions)."""
    a = np.asarray(vals, dtype=np.int16).reshape(nblocks * 8, 16).T
    return np.tile(a, (8, 1))


def _staged2(nodec, nodet, nodesl):
    """Chunked-AllGather row layout: table A = [core-major tiles 0..31],
    table B = [core-major tiles 32..48] (B rows offset by NSA)."""
    r = nodet * P + nodesl
    return np.where(nodet < TCHUNK, nodec * R1 + r,
                    NSA + nodec * (NPAD - R1) + (r - R1))


# ---------------------------------------------------------------- host preprocessing
def _assign_nodes(d_all, N):
    """LPT-assign nodes to C*TILES buckets of <=128 slots, minimizing the
    max per-bucket edge count. Returns per-node (core, tile, slot)."""
    import heapq
    w = np.bincount(d_all, minlength=N)
    nb = C * TILES
    heap = [(0, b) for b in range(nb)]
    heapq.heapify(heap)
    cnt = np.zeros(nb, np.int64)
    nodec = np.empty(N, np.int64)
    nodet = np.empty(N, np.int64)
    nodesl = np.empty(N, np.int64)
    for n in np.argsort(-w, kind="stable"):
        while True:
            wt, b = heapq.heappop(heap)
            if cnt[b] < P:
                break
        nodec[n] = b // TILES
        nodet[n] = b % TILES
        nodesl[n] = cnt[b]
        cnt[b] += 1
        if cnt[b] < P:
            heapq.heappush(heap, (wt + int(w[n]), b))
    return nodec, nodet, nodesl


def _split_blocks(ent, C_, TILES_):
    """Given per-(core,tile) entry dicts with a 'wcls' window class
    (0=table A, 2=table B; forced by the src tile), choose global
    per-tile (BA, BB) block counts feasible for every core and return
    them plus per-core selectors of which entries go to the A blocks."""
    e_ct = np.zeros((C_, TILES_), np.int64)
    a0_ct = np.zeros((C_, TILES_), np.int64)
    fx_ct = np.zeros((C_, TILES_), np.int64)
    for (c, t), (w,) in ((k, (v[-1],)) for k, v in ent.items()):
        e_ct[c, t] = len(w)
        a0_ct[c, t] = int((w == 0).sum())
        fx_ct[c, t] = int((w == 1).sum())
    BA = np.zeros(TILES_, np.int64)
    BB = np.zeros(TILES_, np.int64)
    for t in range(TILES_):
        B = int(max(-(-e_ct[c, t] // P) for c in range(C_)))
        while True:
            cands = []
            for ba in range(0, B + 1):
                bb = B - ba
                ok = all(
                    max(a0_ct[c, t], e_ct[c, t] - P * bb)
                    <= min(a0_ct[c, t] + fx_ct[c, t], P * ba)
                    for c in range(C_))
                if ok:
                    cands.append(ba)
            if cands:
                want = (a0_ct[:, t] + fx_ct[:, t] * 0.5).mean() / P
                BA[t] = min(cands, key=lambda ba: abs(ba - want))
                BB[t] = B - BA[t]
                break
            B += 1

    def isA_for(c, t):
        w = ent[(c, t)][-1]
        lo = max(a0_ct[c, t], e_ct[c, t] - P * BB[t])
        hi = min(a0_ct[c, t] + fx_ct[c, t], P * BA[t])
        kA = int(np.clip(P * BA[t], lo, hi))
        isA = w == 0
        if kA > a0_ct[c, t]:
            isA = isA.copy()
            isA[np.flatnonzero(w == 1)[:kA - a0_ct[c, t]]] = True
        return isA

    return BA, BB, isA_for


def _layout_enc(BA, BB):
    """Encoder block layout: per group, A-runs (tiles in order), B-runs,
    then one self block per tile.  Also numbers gatherable (non-self)
    blocks with contiguous gather positions per group: A first, B after.
    Returns baseA, baseB, selfblk, per-group info, total blocks, gpos
    array and total gather positions."""
    baseA = np.zeros(TILES, np.int64)
    baseB = np.zeros(TILES, np.int64)
    selfblk = np.zeros(TILES, np.int64)
    ginfo = []
    off = 0
    goff = 0
    for g in GROUPS:
        blk0, g0 = off, goff
        for t in g:
            baseA[t] = off
            off += BA[t]
        nbA = off - blk0
        for t in g:
            baseB[t] = off
            off += BB[t]
        nbAB = off - blk0
        for t in g:
            selfblk[t] = off
            off += 1
        gA = int(sum(BA[t] for t in g))
        gAB = int(sum(BA[t] + BB[t] for t in g))
        goff += gAB
        ginfo.append((blk0, int(nbA), int(nbAB), int(off - blk0),
                      g0, gA, gAB))
    gpos = np.full(off, -1, np.int64)
    for gi, g in enumerate(GROUPS):
        blk0, nbA, nbAB, nb, g0, gA, gAB = ginfo[gi]
        for j in range(nbAB):
            gpos[blk0 + j] = g0 + j
    return baseA, baseB, selfblk, ginfo, int(off), gpos, int(goff)


def _layout_dec(DA, DB):
    """Decoder block layout: all A-runs tile-major, then all B-runs."""
    baseA = np.zeros(TILES, np.int64)
    baseB = np.zeros(TILES, np.int64)
    off = 0
    for t in range(TILES):
        baseA[t] = off
        off += DA[t]
    SDA = off
    for t in range(TILES):
        baseB[t] = off
        off += DB[t]
    return baseA, baseB, int(SDA), int(off)


def _preprocess(x, edge_index, W1, b1, W2, b2):
    N = x.shape[0]
    src = edge_index[0].astype(np.int64)
    dst = edge_index[1].astype(np.int64)
    loop = np.arange(N, dtype=np.int64)
    s_all = np.concatenate([src, loop])
    d_all = np.concatenate([dst, loop])
    deg = np.bincount(d_all, minlength=N).astype(np.float64)
    dinv = 1.0 / np.sqrt(deg)
    norm = (dinv[s_all] * dinv[d_all]).astype(np.float32)

    nodec, nodet, nodesl = _assign_nodes(d_all, N)
    staged = _staged2(nodec, nodet, nodesl)

    x16 = x.astype(np.float16)

    def bucket(edst):
        """Group entry indices by (core,tile) of their dst."""
        key = nodec[edst] * TILES + nodet[edst]
        order = np.argsort(key, kind="stable")
        bnd = np.searchsorted(key[order], np.arange(C * TILES + 1))
        out = {}
        for c in range(C):
            for t in range(TILES):
                out[(c, t)] = order[bnd[c * TILES + t]:bnd[c * TILES + t + 1]]
        return out

    # ======== encoder blocks (real edges by dst owner + 1 self block/tile)
    sstg_e = staged[src]
    wclsE = 2 * (sstg_e >= NSA).astype(np.int64)   # A (tile<32) or B, forced
    normE = norm[:N_EDGES]
    buck = bucket(dst)
    ent = {}
    for (c, t), idx in buck.items():
        ent[(c, t)] = (src[idx], sstg_e[idx], nodesl[dst[idx]],
                       normE[idx], wclsE[idx])
    BA, BB, isA_for = _split_blocks(ent, C, TILES)
    baseA, baseB, selfblk, ginfo, SBn, gpos, NG = _layout_enc(BA, BB)

    smat = np.zeros((C, P, SBn * P), dtype=np.float16)
    gidx = np.zeros((C, NG * P), dtype=np.int64)
    xe = np.zeros((C, P, SBn, D_IN), dtype=np.float16)
    for c in range(C):
        for t in range(TILES):
            sraw, ss, sl, nm, w = ent[(c, t)]
            isA = isA_for(c, t)
            for sel, base, wb in ((isA, baseA[t], 0), (~isA, baseB[t], NSA)):
                sraw_s, ss_s, sl_s, nm_s = sraw[sel], ss[sel], sl[sel], nm[sel]
                pos = np.arange(len(ss_s))
                bo = base + pos // P
                lane = pos % P
                smat[c, lane, bo * P + sl_s] = nm_s
                gidx[c, gpos[bo] * P + lane] = ss_s - wb
                xe[c, lane, bo, :] = x16[sraw_s]
    # self blocks: lane=slot=s, S=dinv^2, xe row = x[node]
    dv2 = (dinv * dinv).astype(np.float16)
    smat[nodec, nodesl, selfblk[nodet] * P + nodesl] = dv2
    xe[nodec, nodesl, selfblk[nodet], :] = x16
    gidx16 = np.stack([_wrap16(gidx[c], NG) for c in range(C)])

    # ======== decoder blocks (real edges, by dst owner) ========
    dent = {}
    for (c, t), idx in buck.items():
        dent[(c, t)] = (idx, staged[src[idx]], nodesl[dst[idx]], wclsE[idx])
    DA, DB, disA_for = _split_blocks(
        {k: (v[1], v[2], v[3]) for k, v in dent.items()}, C, TILES)
    dbaseA, dbaseB, SDA, SD = _layout_dec(DA, DB)

    # decoder chunks (A-range then B-range, GMAX blocks each)
    chunks = []
    for r0, r1 in ((0, SDA), (SDA, SD)):
        for c0 in range(r0, r1, GMAX):
            chunks.append((c0, min(c0 + GMAX, r1)))
    NCH = len(chunks)
    chunk_of = np.zeros(SD, np.int64)
    off_of = np.zeros(SD, np.int64)
    for i, (c0, c1) in enumerate(chunks):
        chunk_of[c0:c1] = i
        off_of[c0:c1] = np.arange(c1 - c0)

    s01T = np.zeros((C, P, SD * P), dtype=np.float16)
    didx = np.zeros((C, SD * P), dtype=np.int64)
    perm = np.full(N_EDGES, -1, np.int64)     # edge -> flat logit position
    for c in range(C):
        for t in range(TILES):
            eid, ss, dsl, w = dent[(c, t)]
            isA = disA_for(c, t)
            for sel, base, wb in ((isA, dbaseA[t], 0), (~isA, dbaseB[t], NSA)):
                eid_s, ss_s, dsl_s = eid[sel], ss[sel], dsl[sel]
                pos = np.arange(len(eid_s))
                bo = base + pos // P
                lane = pos % P
                s01T[c, dsl_s, bo * P + lane] = 1.0
                didx[c, bo * P + lane] = ss_s - wb
                perm[eid_s] = (chunk_of[bo] * (GMAX * P)
                               + off_of[bo] * P + lane)
    didx16 = np.stack([_wrap16(didx[c], SD) for c in range(C)])

    # block -> owning tile (for zloc expansion)
    btile = np.zeros(SD, np.int64)
    for t in range(TILES):
        btile[dbaseA[t]:dbaseA[t] + DA[t]] = t
        btile[dbaseB[t]:dbaseB[t] + DB[t]] = t

    ecore_of_edge = nodec[dst]

    shared = {
        "w1": np.ascontiguousarray(W1.astype(np.float16)),
        "w2": np.ascontiguousarray(W2.astype(np.float16)),
        "ident": np.eye(P, dtype=np.float16),
        "b1c": np.ascontiguousarray(b1.astype(np.float32).reshape(D_H, 1)),
        "b2r": np.ascontiguousarray(
            np.broadcast_to(b2.astype(np.float32), (P, D_OUT))),
    }
    in_maps = []
    for c in range(C):
        m = dict(shared)
        m["xe"] = np.ascontiguousarray(xe[c].reshape(P, SBn * D_IN))
        m["smat"] = np.ascontiguousarray(smat[c])
        m["gidx"] = np.ascontiguousarray(gidx16[c])
        m["s01"] = np.ascontiguousarray(s01T[c])
        m["didx"] = np.ascontiguousarray(didx16[c])
        in_maps.append(m)

    spec = dict(BA=tuple(int(v) for v in BA), BB=tuple(int(v) for v in BB),
                baseA=tuple(int(v) for v in baseA),
                baseB=tuple(int(v) for v in baseB),
                selfblk=tuple(int(v) for v in selfblk),
                ginfo=tuple(ginfo), SBn=SBn, NG=NG,
                gpos=tuple(int(v) for v in gpos),
                DA=tuple(int(v) for v in DA), DB=tuple(int(v) for v in DB),
                dbaseA=tuple(int(v) for v in dbaseA),
                dbaseB=tuple(int(v) for v in dbaseB),
                SD=SD, SDA=SDA, NCH=NCH, chunks=tuple(chunks),
                btile=tuple(int(v) for v in btile))
    return in_maps, spec, (perm, ecore_of_edge)


# ---------------------------------------------------------------- device program
def _build(spec):
    BA, BB = spec["BA"], spec["BB"]
    baseA, baseB = spec["baseA"], spec["baseB"]
    selfblk = spec["selfblk"]
    ginfo, SBn, NG = spec["ginfo"], spec["SBn"], spec["NG"]
    gpos = spec["gpos"]
    DA, DB = spec["DA"], spec["DB"]
    dbaseA, dbaseB = spec["dbaseA"], spec["dbaseB"]
    SD, SDA, NCH = spec["SD"], spec["SDA"], spec["NCH"]
    chunks, btile = spec["chunks"], spec["btile"]

    nc = bacc.Bacc("TRN2", target_bir_lowering=False, debug=False,
                   enable_asserts=False, num_devices=C, num_swdge_queues=NQ)

    xe_d = nc.dram_tensor("xe", [P, SBn * D_IN], F16, kind="ExternalInput")
    w1_d = nc.dram_tensor("w1", [D_IN, D_H], F16, kind="ExternalInput")
    w2_d = nc.dram_tensor("w2", [D_H, D_OUT], F16, kind="ExternalInput")
    ident_d = nc.dram_tensor("ident", [P, P], F16, kind="ExternalInput")
    b1c_d = nc.dram_tensor("b1c", [D_H, 1], F32, kind="ExternalInput")
    b2r_d = nc.dram_tensor("b2r", [P, D_OUT], F32, kind="ExternalInput")
    smat_d = nc.dram_tensor("smat", [P, SBn * P], F16, kind="ExternalInput")
    gidx_d = nc.dram_tensor("gidx", [P, NG * 8], I16, kind="ExternalInput")
    s01_d = nc.dram_tensor("s01", [P, SD * P], F16, kind="ExternalInput")
    didx_d = nc.dram_tensor("didx", [P, SD * 8], I16, kind="ExternalInput")
    logits_d = nc.dram_tensor("logits", [NCH, GMAX * P], F32,
                              kind="ExternalOutput")
    debug = bool(int(os.environ.get("KERNEL_DEBUG_DUMP", "0")))
    if debug:
        h2dump_d = nc.dram_tensor("h2dump", [NS, P], F16,
                                  kind="ExternalOutput")
        zdump_d = nc.dram_tensor("zdump", [NS, P], F16, kind="ExternalOutput")

    rg = [list(range(C))]
    qctr = [0]

    def nextq():
        qctr[0] += 1
        return qctr[0] % NQ

    def chain(t):
        """Per-tile matmul chain: edge blocks then the self block."""
        out = [baseA[t] + b for b in range(BA[t])]
        out += [baseB[t] + b for b in range(BB[t])]
        out.append(selfblk[t])
        return out

    from contextlib import ExitStack
    with tile.TileContext(nc) as tc:
        with ExitStack() as stack:
            _p = lambda **kw: stack.enter_context(tc.tile_pool(**kw))
            constp = _p(name="const", bufs=1)
            metap = _p(name="meta", bufs=1)
            sp = _p(name="sblk", bufs=2)
            xgp = _p(name="xg", bufs=2)
            xaggp = _p(name="xagg", bufs=2)
            kxnp = _p(name="kxn", bufs=2)
            h1rp = _p(name="h1r", bufs=2)
            h2sp = _p(name="h2s", bufs=2)
            h2resp = _p(name="h2res", bufs=1)
            hgp = _p(name="hg", bufs=2)
            zlocp = _p(name="zloc", bufs=1)
            zsp = _p(name="zs", bufs=3)
            s01p = _p(name="s01c", bufs=2)
            prp = _p(name="pr", bufs=2)
            lrp = _p(name="lr", bufs=2)
            pacc = _p(name="pacc", bufs=2, space="PSUM")
            php = _p(name="ph", bufs=2, space="PSUM")
            pzp = _p(name="pz", bufs=2, space="PSUM")
            dramp = _p(name="dram", bufs=1, space="DRAM")

            # ---- persistent tables
            w1sb = []
            for k, (k0, kw) in enumerate(KCH):
                t_ = constp.tile([kw, D_H], F16, name=f"w1sb{k}",
                                 tag=f"w1sb{k}")
                nc.scalar.dma_start(out=t_[:], in_=w1_d[k0:k0 + kw, :])
                w1sb.append(t_)
            w2sb = []
            b1sb = []
            for m, (m0, mw) in enumerate(MCH):
                t_ = constp.tile([mw, D_OUT], F16, name=f"w2sb{m}",
                                 tag=f"w2sb{m}")
                nc.scalar.dma_start(out=t_[:], in_=w2_d[m0:m0 + mw, :])
                w2sb.append(t_)
                bt = constp.tile([mw, 1], F32, name=f"b1sb{m}", tag=f"b1sb{m}")
                nc.scalar.dma_start(out=bt[:], in_=b1c_d[m0:m0 + mw, :])
                b1sb.append(bt)
            idn = constp.tile([P, P], F16, name="idn", tag="idn")
            nc.scalar.dma_start(out=idn[:], in_=ident_d[:, :])
            b2sb = constp.tile([P, D_OUT], F32, name="b2sb", tag="b2sb")
            nc.scalar.dma_start(out=b2sb[:], in_=b2r_d[:, :])
            ones = constp.tile([D_OUT, 1], F16, name="ones", tag="ones")
            nc.vector.memset(ones[:], 1.0)
            gidx_sb = metap.tile([P, max(NG, SD) * 8], I16, name="gidx_sb",
                                 tag="gidx")
            nc.scalar.dma_start(out=gidx_sb[:, 0:NG * 8], in_=gidx_d[:, :])

            h2pad = dramp.tile([NPAD, P], F16, name="h2pad", tag="h2pad")
            h2fullA = dramp.tile([NSA, P], F16, name="h2fullA", tag="h2fullA",
                                 addr_space="Shared")
            h2fullB = dramp.tile([NSB, P], F16, name="h2fullB", tag="h2fullB",
                                 addr_space="Shared")
            zpad = dramp.tile([NPAD, P], F16, name="zpad", tag="zpad")
            zfullA = dramp.tile([NSA, P], F16, name="zfullA", tag="zfullA",
                                addr_space="Shared")
            zfullB = dramp.tile([NSB, P], F16, name="zfullB", tag="zfullB",
                                addr_space="Shared")

            def load_group(g, s_pool, s_eng, with_x):
                blk0, nbA, nbAB, nb = ginfo[g][:4]
                st = s_pool.tile([P, nb, P], F16, name="s_sb", tag="s_sb")
                s_eng.dma_start(out=st[:],
                                in_=smat_d[:, blk0 * P:(blk0 + nb) * P])
                if not with_x:
                    return st, None
                xg = xgp.tile([P, nb, D_IN], F16, name="xg", tag="xg")
                half = nb // 2
                nc.scalar.dma_start(
                    out=xg[:, 0:half, :],
                    in_=xe_d[:, blk0 * D_IN:(blk0 + half) * D_IN])
                nc.sync.dma_start(
                    out=xg[:, half:nb, :],
                    in_=xe_d[:, (blk0 + half) * D_IN:(blk0 + nb) * D_IN])
                return st, xg

            # ---- layer 1 (xe streamed f16)
            h2res = []
            nxt = load_group(0, sp, nc.gpsimd, True)
            for g, tlist in enumerate(GROUPS):
                blk0, nbA, nbAB, nb = ginfo[g][:4]
                gw = len(tlist) * P
                s_sb, xg = nxt
                if g + 1 < len(GROUPS):
                    nxt = load_group(g + 1, sp, nc.gpsimd, True)
                kxn = kxnp.tile([P, 5, gw], F16, name="kxn", tag="kxn")
                for j, t in enumerate(tlist):
                    acc = pacc.tile([P, D_IN], F32, name="acc", tag="acc")
                    ch_ = chain(t)
                    for i, o in enumerate(ch_):
                        jl = o - blk0
                        st0, st1 = i == 0, i == len(ch_) - 1
                        nc.tensor.matmul(
                            acc[:, 0:512], lhsT=s_sb[:, jl, :],
                            rhs=xg[:, jl, 0:512], start=st0, stop=False)
                        nc.tensor.matmul(
                            acc[:, 512:D_IN], lhsT=s_sb[:, jl, :],
                            rhs=xg[:, jl, 512:D_IN], start=st0, stop=st1)
                    xaggsb = xaggp.tile([P, D_IN], F16, name="xaggsb",
                                        tag="xaggsb")
                    nc.scalar.copy(out=xaggsb[:], in_=acc[:])
                    for k, (k0, kw) in enumerate(KCH):
                        tp = pzp.tile([P, P], F16, name="tp", tag="pz2")
                        nc.tensor.transpose(out=tp[:kw, :],
                                            in_=xaggsb[:, k0:k0 + kw],
                                            identity=idn[:])
                        nc.scalar.copy(
                            out=kxn[0:kw, k, j * P:(j + 1) * P],
                            in_=tp[:kw, :])
                # GEMM1 + relu (feat-major), GEMM2
                h1r = h1rp.tile([P, 5, gw], F16, name="h1r", tag="h1r")
                for m, (m0, mw) in enumerate(MCH):
                    hp = php.tile([P, gw], F32, name="hp", tag="hp")
                    for k, (k0, kw) in enumerate(KCH):
                        nc.tensor.matmul(hp[:mw, :],
                                         lhsT=w1sb[k][:, m0:m0 + mw],
                                         rhs=kxn[0:kw, k, :],
                                         start=(k == 0), stop=(k == 4))
                    nc.scalar.activation(out=h1r[:mw, m, :], in_=hp[:mw, :],
                                         func=mybir.ActivationFunctionType.Relu,
                                         bias=b1sb[m][:], scale=1.0)
                h2p = php.tile([P, gw], F32, name="h2p", tag="hp")
                for m, (m0, mw) in enumerate(MCH):
                    nc.tensor.matmul(h2p[:D_OUT, :], lhsT=w2sb[m][:],
                                     rhs=h1r[0:mw, m, :],
                                     start=(m == 0), stop=(m == 4))
                h2sb = h2sp.tile([D_OUT, gw], F16, name="h2sb", tag="h2sb")
                nc.scalar.copy(out=h2sb[:], in_=h2p[:D_OUT, :])
                for j, t in enumerate(tlist):
                    h2row = h2resp.tile([P, D_OUT], F16, name=f"h2r{t}",
                                        tag=f"h2r{t}")
                    h2res.append(h2row)
                    nc.scalar.dma_start(out=h2row[:],
                                        in_=h2sb[:, j * P:(j + 1) * P],
                                        transpose=True)
                    nc.scalar.dma_start(
                        out=h2pad[t * P:(t + 1) * P, 0:D_OUT], in_=h2row[:])
                if g == NGROUP_C1 - 1:
                    nc.gpsimd.collective_compute(
                        "AllGather", mybir.AluOpType.bypass,
                        replica_groups=rg, ins=[h2pad[0:R1, :].opt()],
                        outs=[h2fullA[:, :].opt()])
            nc.gpsimd.collective_compute(
                "AllGather", mybir.AluOpType.bypass, replica_groups=rg,
                ins=[h2pad[R1:NPAD, :].opt()],
                outs=[h2fullB[:, :].opt()])
            if debug:
                nc.sync.dma_start(out=h2dump_d[0:NSA, :], in_=h2fullA[:])
                nc.sync.dma_start(out=h2dump_d[NSA:NS, :], in_=h2fullB[:])

            # ---- layer 2 (batched gathers; self rows from SBUF h2res)
            zloc = []
            for t in range(TILES):
                zt = zlocp.tile([P, D_OUT], F16, name=f"zloc{t}",
                                tag=f"zloc{t}")
                zloc.append(zt)
            for g, tlist in enumerate(GROUPS):
                blk0, nbA, nbAB, nb, g0, gA, gAB = ginfo[g]
                s_sb, _ = load_group(g, sp, nc.scalar, False)
                ng = gAB
                hg = hgp.tile([P, ng, P], F16, name="hg", tag="hg")
                runs = [(0, gA, h2fullA[:, :])] if gA else []
                if gAB - gA:
                    runs.append((gA, gAB, h2fullB[:, :]))
                for r0, r1, tab in runs:
                    for c0 in range(r0, r1, GMAX):
                        c1 = min(c0 + GMAX, r1)
                        nc.gpsimd.dma_gather(
                            out_ap=hg[:, c0:c1, :], in_ap=tab,
                            idxs_ap=gidx_sb[:, (g0 + c0) * 8:(g0 + c1) * 8],
                            num_idxs=(c1 - c0) * P, num_idxs_reg=(c1 - c0) * P,
                            elem_size=P, queue_num=nextq())
                for j, t in enumerate(tlist):
                    acc2 = pzp.tile([P, D_OUT], F32, name="acc2", tag="pz2")
                    ents = ([(baseA[t] + b) for b in range(BA[t])]
                            + [(baseB[t] + b) for b in range(BB[t])])
                    for i, o in enumerate(ents):
                        lg_ = gpos[o] - g0
                        nc.tensor.matmul(
                            acc2[:], lhsT=s_sb[:, o - blk0, :],
                            rhs=hg[:, lg_, 0:D_OUT],
                            start=(i == 0), stop=False)
                    nc.tensor.matmul(
                        acc2[:], lhsT=s_sb[:, selfblk[t] - blk0, :],
                        rhs=h2res[t][:, 0:D_OUT], start=False, stop=True)
                    nc.vector.tensor_add(out=zloc[t][:], in0=acc2[:],
                                         in1=b2sb[:])
                    nc.sync.dma_start(
                        out=zpad[t * P:(t + 1) * P, 0:D_OUT], in_=zloc[t][:])
                if g == NGROUP_C1 - 1:
                    nc.gpsimd.collective_compute(
                        "AllGather", mybir.AluOpType.bypass,
                        replica_groups=rg, ins=[zpad[0:R1, :].opt()],
                        outs=[zfullA[:, :].opt()])
            nc.gpsimd.collective_compute(
                "AllGather", mybir.AluOpType.bypass, replica_groups=rg,
                ins=[zpad[R1:NPAD, :].opt()],
                outs=[zfullB[:, :].opt()])
            if debug:
                nc.sync.dma_start(out=zdump_d[0:NSA, :], in_=zfullA[:])
                nc.sync.dma_start(out=zdump_d[NSA:NS, :], in_=zfullB[:])

            # ---- decoder: zsT gathered transposed, zdeT batched from zloc
            didx_sb = metap.tile([P, max(NG, SD) * 8], I16, name="didx_sb",
                                 tag="gidx")
            nc.scalar.dma_start(out=didx_sb[:, 0:SD * 8], in_=didx_d[:, :])
            for i, (c0, c1) in enumerate(chunks):
                ch = c1 - c0
                W = ch * P
                tab = zfullA[:, :] if c1 <= SDA else zfullB[:, :]
                zsT = zsp.tile([P, 1, GMAX * P], F16, name="zsT", tag="zsT")
                # transposed gathers hang at num_idxs=1024; cap at 512
                for s0 in range(0, ch, TGMAX):
                    s1 = min(s0 + TGMAX, ch)
                    nc.gpsimd.dma_gather(
                        out_ap=zsT[:, :, s0 * P:s1 * P], in_ap=tab,
                        idxs_ap=didx_sb[:, (c0 + s0) * 8:(c0 + s1) * 8],
                        num_idxs=(s1 - s0) * P, num_idxs_reg=(s1 - s0) * P,
                        elem_size=P, transpose=True, queue_num=nextq())
                s01c = s01p.tile([P, GMAX * P], F16, name="s01c", tag="s01c")
                nc.scalar.dma_start(out=s01c[:, 0:W],
                                    in_=s01_d[:, c0 * P:c1 * P])
                prT = prp.tile([D_OUT, GMAX * P], F16, name="prT", tag="prT")
                # tile-runs within the chunk, split at 4 blocks (PSUM bank)
                o = c0
                while o < c1:
                    t = btile[o]
                    o1 = o
                    while o1 < c1 and btile[o1] == t and o1 - o < 4:
                        o1 += 1
                    w = (o1 - o) * P
                    zdeT = pzp.tile([D_OUT, 4 * P], F32, name="zdeT",
                                    tag="pz2")
                    nc.tensor.matmul(
                        zdeT[:, 0:w], lhsT=zloc[t][:, :],
                        rhs=s01c[:, (o - c0) * P:(o1 - c0) * P],
                        start=True, stop=True)
                    nc.vector.tensor_mul(
                        out=prT[:, (o - c0) * P:(o1 - c0) * P],
                        in0=zsT[0:D_OUT, 0, (o - c0) * P:(o1 - c0) * P],
                        in1=zdeT[:, 0:w])
                    o = o1
                lrow = lrp.tile([1, GMAX * P], F32, name="lrow", tag="lrow")
                for h0 in range(0, W, 512):
                    hw = min(512, W - h0)
                    lgp = php.tile([1, 512], F32, name="lgp", tag="hp")
                    nc.tensor.matmul(lgp[:, 0:hw], lhsT=ones[:],
                                     rhs=prT[:, h0:h0 + hw],
                                     start=True, stop=True)
                    nc.scalar.copy(out=lrow[:, h0:h0 + hw], in_=lgp[:, 0:hw])
                nc.sync.dma_start(out=logits_d[i:i + 1, 0:W],
                                  in_=lrow[:, 0:W])

    nc.compile()
    return nc


# ---------------------------------------------------------------- entry point
_CACHE = {}


def kernel(x, edge_index, W1, b1, W2, b2):
    x = np.asarray(x)
    edge_index = np.asarray(edge_index)
    in_maps, spec, (perm, ecore) = _preprocess(
        x, edge_index, np.asarray(W1), np.asarray(b1), np.asarray(W2),
        np.asarray(b2))
    key = (spec["BA"], spec["BB"], spec["DA"], spec["DB"])
    if key not in _CACHE:
        _CACHE[key] = _build(spec)
    nc = _CACHE[key]
    res = bass_utils.run_bass_kernel_spmd(nc, in_maps, core_ids=list(range(C)))
    out = np.empty(N_EDGES, dtype=np.float32)
    for c in range(C):
        lg = res.results[c]["logits"].reshape(-1)     # [NCH*GMAX*P]
        mine = np.flatnonzero(ecore == c)
        out[mine] = lg[perm[mine]]
    return out
